# revision 2
# baseline (speedup 1.0000x reference)
"""PatternMemory kernel for 8 Trainium2 NeuronCores.

Math (B=8, T=1024, C=1024, P=100):
  ctx_h = context @ W1[:C]                   (B, C)
  trg_h = triggers @ W1[C:]                  (P, C)
  h = relu(ctx_h[:,None,:] + trg_h[None,:,:] + b1)
  logits = h @ W2 + b2[0]                    (B, P)
  scores = sigmoid(logits).mean(axis=0)      (P,)
  w = where(scores > 0.5, scores * conf, 0)
  out = attention_scores + 0.1 * einsum("p,pij->ij", w, biases)

Sharding: core r owns rows [128r, 128(r+1)) of the (T, T) plane; every
core does the full MLP redundantly (no collectives).

This problem is DMA-bound: 65.9MB of reads + 4.2MB of writes per core
at the measured ~410GB/s aggregate = ~165us floor.  Everything else is
scheduled to keep the DMA stream saturated from t=0 to the end:

- ALL loads go on the ONE sync-queue in explicit order: small consts,
  W1 (16 tiles - the MLP gates everything), then the 100 bias planes
  interleaved (pe/dve alternating for the first 50 so the PE block
  finishes mid-stream), with the 8 attn planes inserted before the
  last 6 dve planes.  (The previous version put PE planes on the
  scalar queue "behind a gate"; the Tile scheduler does not respect
  queue program order, so those DMAs ran at t=11us, halved the W1
  bandwidth, and pushed weights-ready to ~89us causing a ~20us DMA
  stall when the 29-slot ring filled.)
- One shared 40-slot [128,1024] ring holds W1 + bias + attn tiles:
  big enough to absorb the backlog that accumulates until the MLP
  weights are ready (~50us), so the DMA never stalls.
- The rel-err gate is decided at catastrophic-cancellation outputs
  where |attn| ~ |0.1*acc|; passing requires the accumulation to stay
  ULP-correlated with the reference's serial p-order einsum.  The
  pattern partition is therefore kept EXACTLY as the passing baseline:
  PE accumulates planes 0..24 (diag-stationary matmuls, PSUM chain in
  ascending p), DVE runs three fused chains 25..49 / 50..74 / 75..99,
  merged as (((Q1+Q0)+Q2)+Q3).  The merges are slotted mid-stream
  (after planes 55 / 80) so only the final (+Q3) merge is in the tail.
- MLP phase B runs its relus on DVE as fused (add,max) tensor_scalar
  ops (numerically identical: round(a+b) then max with 0), which is
  ~2x faster than the 64 tiny Act ops and pulls weights-ready down.
- Tail: the 8 attn adds are split DVE (b<4) / GpSimd (b>=4) with the
  store for each plane issued as soon as its add completes, so the
  4.2MB write drain starts ~2us after the last bias byte arrives.
"""

import numpy as np
import bass_rust

from concourse import bass, mybir
from concourse.bass_utils import run_bass_kernel_spmd
from concourse.tile import TileContext

B, T, C, P = 8, 1024, 1024, 100
NCORES = 8
ROWS = T // NCORES  # 128 rows of the (T, T) plane per core
FP32 = mybir.dt.float32
AF = mybir.ActivationFunctionType
ALU = mybir.AluOpType

SIM_THRESHOLD = 0.5
LAMBDA = 0.1

POOL_BUFS = 40      # shared rotating [128,1024] slots (w1 / bias / attn)
NKT = 16            # 2C/128 contraction tiles
NST = 108           # combined stationary width: 100 trig + 8 ctx
NPE = 25            # patterns 0..NPE-1 accumulate on the PE

_NC_CACHE = {}


def _build_nc() -> bass.Bass:
    nc = bass.Bass("TRN2", target_bir_lowering=False, debug=False,
                   num_devices=NCORES)

    bias_s = nc.dram_tensor("bias_s", (P - NPE, ROWS, T), FP32,
                            kind="ExternalInput").ap()
    biasp_s = nc.dram_tensor("biasp_s", (NPE, ROWS, T), FP32,
                             kind="ExternalInput").ap()
    attn_s = nc.dram_tensor("attn_s", (B, ROWS, T), FP32, kind="ExternalInput").ap()
    # packed W1: [r, kt*1024 + ch*512 + c'] = W1[kt*128 + r, ch*512 + c']
    w1p = nc.dram_tensor("w1p", (128, NKT * 1024), FP32, kind="ExternalInput").ap()
    # combined stationary: [r, kt*108 + j]; kt<8: cols 100..108 = ctxT,
    # kt>=8: cols 0..100 = trigT; rest zero.
    combp = nc.dram_tensor("combp", (128, NKT * NST), FP32, kind="ExternalInput").ap()
    b1c = nc.dram_tensor("b1c", (128, 8), FP32, kind="ExternalInput").ap()
    w2r = nc.dram_tensor("w2r", (128, 8), FP32, kind="ExternalInput").ap()
    conf = nc.dram_tensor("conf", (1, P), FP32, kind="ExternalInput").ap()
    b2 = nc.dram_tensor("b2", (1, 1), FP32, kind="ExternalInput").ap()
    out_s = nc.dram_tensor("out_s", (B, ROWS, T), FP32, kind="ExternalOutput").ap()

    with TileContext(nc) as tc:
        with tc.tile_pool(name="const", bufs=1) as const_pool, \
             tc.tile_pool(name="mlp", bufs=1) as mlp_pool, \
             tc.tile_pool(name="rot", bufs=2) as rot_pool, \
             tc.tile_pool(name="diag", bufs=4) as diag_pool, \
             tc.tile_pool(name="small", bufs=1) as small_pool, \
             tc.tile_pool(name="psA", bufs=1, space="PSUM") as psA, \
             tc.tile_pool(name="psB", bufs=1, space="PSUM") as psB, \
             tc.tile_pool(name="psD", bufs=2, space="PSUM") as psD, \
             tc.tile_pool(name="big", bufs=POOL_BUFS) as big_pool, \
             tc.tile_pool(name="accp", bufs=1) as acc_pool:

            # ---- small consts first on the Sync queue ----
            combt = const_pool.tile([128, NKT * NST], FP32, tag="combp",
                                    name="combt")
            nc.sync.dma_start(out=combt, in_=combp)
            b1t = const_pool.tile([128, 8], FP32, tag="b1c", name="b1t")
            nc.sync.dma_start(out=b1t, in_=b1c)
            w2t = const_pool.tile([128, 8], FP32, tag="w2r", name="w2t")
            nc.sync.dma_start(out=w2t, in_=w2r)
            conft = const_pool.tile([1, P], FP32, tag="conf", name="conft")
            nc.sync.dma_start(out=conft, in_=conf)
            b2t = const_pool.tile([1, 1], FP32, tag="b2", name="b2t")
            nc.sync.dma_start(out=b2t, in_=b2)

            # ---- W1 as 16 [128,1024] slices (kt-major) ----
            w1_slots = [big_pool.tile([128, T], FP32, tag="big", name=f"w1s{i}")
                        for i in range(NKT)]
            for kt in range(NKT):
                nc.sync.dma_start(out=w1_slots[kt],
                                  in_=w1p[:, kt * 1024:(kt + 1) * 1024])

            # ---- the full plane stream, single queue, explicit order:
            # (pe0,dve25,pe1,dve26,...,pe24,dve49), dve50..93,
            # attn0..7, dve94..99.  PE planes land early so the PE
            # PSUM block finishes mid-stream; attn lands before the
            # last dve planes so the tail adds are never attn-gated.
            stream = []
            for g in range(NPE):
                stream.append(("pe", g))
                stream.append(("dve", NPE + g))
            stream += [("dve", p) for p in range(50, 94)]
            stream += [("attn", b) for b in range(B)]
            stream += [("dve", p) for p in range(94, P)]

            bias_tiles = {}
            attns = [None] * B
            for kind, idx in stream:
                if kind == "attn":
                    at = big_pool.tile([128, T], FP32, tag="big",
                                       name=f"attn{idx}")
                    nc.sync.dma_start(out=at, in_=attn_s[idx])
                    attns[idx] = at
                else:
                    bt = big_pool.tile([128, T], FP32, tag="big",
                                       name=f"bias{idx}")
                    if idx < NPE:
                        nc.sync.dma_start(out=bt, in_=biasp_s[idx])
                    else:
                        nc.sync.dma_start(out=bt, in_=bias_s[idx - NPE])
                    bias_tiles[idx] = bt

            ones = const_pool.tile([1, 128], FP32, tag="ones", name="ones")
            nc.vector.memset(ones, 1.0)
            ident = const_pool.tile([128, 128], FP32, tag="ident", name="ident")
            from concourse.masks import make_identity
            make_identity(nc, ident)

            # ---- phase A: one PSUM tile holds trg_hT rows 0..99 and
            # ctx_hT rows 100..107, accumulated over all 16 kt ----
            ps_comb = psA.tile([NST, C], FP32, tag="comb", name="ps_comb")
            comb_hsb = mlp_pool.tile([NST, C], FP32, tag="comb_hsb",
                                     name="comb_hsb")
            log_a = psB.tile([1, 512], FP32, tag="log_a", name="log_a")
            log_b = psB.tile([1, 288], FP32, tag="log_b", name="log_b")

            for kt in range(NKT):
                stat = combt[:, kt * NST:(kt + 1) * NST]
                for ch in range(2):
                    osl = slice(ch * 512, (ch + 1) * 512)
                    nc.tensor.matmul(ps_comb[:, osl], lhsT=stat,
                                     rhs=w1_slots[kt][:, ch * 512:(ch + 1) * 512],
                                     start=(kt == 0), stop=(kt == NKT - 1))
            nc.scalar.activation(out=comb_hsb, in_=ps_comb, func=AF.Copy)

            # ---- phase B: per 128-col tile, transpose to [c, (trig|ctx)]
            # layout, relu-bias per b on DVE, W2 contraction ----
            for ct in range(8):
                csl = slice(ct * 128, (ct + 1) * 128)
                tp = psD.tile([128, NST], FP32, tag="tp", name=f"tp_{ct}")
                nc.tensor.transpose(tp, comb_hsb[:, csl], ident[0:NST, 0:NST])
                # bvs[c, b] = ctx_hT[c, b] + b1[c]
                bvs = rot_pool.tile([128, B], FP32, tag="bvs", name=f"bvs_{ct}")
                nc.vector.tensor_scalar(out=bvs, in0=tp[:, 100:108],
                                        scalar1=b1t[:, ct:ct + 1],
                                        scalar2=None, op0=ALU.add)
                hT = rot_pool.tile([128, B * P], FP32, tag="hT", name=f"hT_{ct}")
                # relu(trg_hT + bvs[:,b]): fused round(a+b) then max(.,0)
                for b in range(B):
                    nc.vector.tensor_scalar(out=hT[:, b * P:(b + 1) * P],
                                            in0=tp[:, 0:P],
                                            scalar1=bvs[:, b:b + 1],
                                            scalar2=0.0,
                                            op0=ALU.add, op1=ALU.max)
                nc.tensor.matmul(log_a, lhsT=w2t[:, ct:ct + 1],
                                 rhs=hT[:, 0:512],
                                 start=(ct == 0), stop=(ct == 7))
                nc.tensor.matmul(log_b, lhsT=w2t[:, ct:ct + 1],
                                 rhs=hT[:, 512:800],
                                 start=(ct == 0), stop=(ct == 7))

            # ---- scores -> weights (tiny, [1, *] on one partition) ----
            sig = small_pool.tile([1, B * P], FP32, tag="sig", name="sig")
            nc.scalar.activation(out=sig[:, 0:512], in_=log_a,
                                 func=AF.Sigmoid, bias=b2t[:, 0:1])
            nc.scalar.activation(out=sig[:, 512:800], in_=log_b,
                                 func=AF.Sigmoid, bias=b2t[:, 0:1])
            ssum = small_pool.tile([1, P], FP32, tag="ssum", name="ssum")
            nc.vector.tensor_add(out=ssum, in0=sig[:, 0:P], in1=sig[:, P:2 * P])
            for b in range(2, B):
                nc.vector.tensor_add(out=ssum, in0=ssum,
                                     in1=sig[:, b * P:(b + 1) * P])
            scores = small_pool.tile([1, P], FP32, tag="scores", name="scores")
            nc.vector.tensor_scalar_mul(out=scores, in0=ssum, scalar1=1.0 / B)
            mask = small_pool.tile([1, P], FP32, tag="mask", name="mask")
            nc.vector.tensor_scalar(out=mask, in0=scores, scalar1=SIM_THRESHOLD,
                                    scalar2=None, op0=ALU.is_gt)
            sc_conf = small_pool.tile([1, P], FP32, tag="sc_conf", name="sc_conf")
            nc.vector.tensor_mul(out=sc_conf, in0=scores, in1=conft)
            # w = (scores * conf * LAMBDA) * mask   (LAMBDA folded in here)
            w_vec = small_pool.tile([1, P], FP32, tag="w_vec", name="w_vec")
            nc.vector.scalar_tensor_tensor(out=w_vec, in0=sc_conf, scalar=LAMBDA,
                                           in1=mask, op0=ALU.mult, op1=ALU.mult)
            # broadcast w to all 128 partitions via rank-1 matmul
            wbc = psD.tile([128, P], FP32, tag="tp", name="wbc")
            nc.tensor.matmul(wbc, lhsT=ones, rhs=w_vec, start=True, stop=True)
            wsb = small_pool.tile([128, P], FP32, tag="wsb", name="wsb")
            nc.scalar.activation(out=wsb, in_=wbc, func=AF.Copy)

            # ---- stream phase ----
            # PE: patterns 0..24 PSUM-accumulated in ascending p; each
            # plane is two [128,512] matmuls with stationary diag(w[p]).
            ps_acc = psA.tile([128, T], FP32, tag="pacc", name="ps_acc")
            for p in range(NPE):
                dg = diag_pool.tile([128, 128], FP32, tag="dg", name=f"dg{p}")
                nc.scalar.activation(out=dg, in_=ident, func=AF.Copy,
                                     scale=wsb[:, p:p + 1])
                for ch in range(2):
                    osl = slice(ch * 512, (ch + 1) * 512)
                    nc.tensor.matmul(ps_acc[:, osl], lhsT=dg,
                                     rhs=bias_tiles[p][:, osl],
                                     start=(p == 0), stop=(p == NPE - 1))

            # DVE: three fused chains over contiguous blocks, merged
            # progressively (((Q0+Q1)+Q2)+Q3).  The (Q1+Q0) merge runs
            # after plane 55 (PE block + Q1 chain are both long done by
            # then), (+Q2) after plane 80, so only the final (+Q3)
            # merge is in the tail.  Merges write accs[0]/accs[1], not
            # the live chain accumulator, so values are identical to
            # the baseline tree.
            bounds = [NPE, 50, 75, P]
            accs = [acc_pool.tile([128, T], FP32, tag=f"ac{c}", name=f"ac{c}")
                    for c in range(3)]
            for ci in range(3):
                for p in range(bounds[ci], bounds[ci + 1]):
                    bt = bias_tiles[p]
                    w_ap = wsb[:, p:p + 1]
                    if p == bounds[ci]:
                        nc.vector.tensor_scalar_mul(out=accs[ci], in0=bt,
                                                    scalar1=w_ap)
                    else:
                        nc.vector.scalar_tensor_tensor(out=accs[ci], in0=bt,
                                                       scalar=w_ap,
                                                       in1=accs[ci],
                                                       op0=ALU.mult,
                                                       op1=ALU.add)
                    if p == 55:
                        nc.vector.tensor_add(out=accs[0], in0=accs[0],
                                             in1=ps_acc)
                    elif p == 80:
                        nc.vector.tensor_add(out=accs[0], in0=accs[0],
                                             in1=accs[1])

            # ---- tail: final merge, attn adds (DVE b<4, GpSimd b>=4),
            # store each plane as soon as its add is done ----
            nc.vector.tensor_add(out=accs[0], in0=accs[0], in1=accs[2])
            for b in range(B):
                eng = nc.vector if b < 4 else nc.gpsimd
                eng.tensor_add(out=attns[b], in0=attns[b], in1=accs[0])
                nc.sync.dma_start(out=out_s[b], in_=attns[b])

    # TRN2 matmul supports only one embedded semaphore wait; split the
    # extras onto InstEventSemaphore instructions (same pass Bacc runs).
    bass_rust.generate_event_semaphores(nc)
    return nc


def _get_nc() -> bass.Bass:
    if "nc" not in _NC_CACHE:
        _NC_CACHE["nc"] = _build_nc()
    return _NC_CACHE["nc"]


def _prep_in_maps(attention_scores, context, triggers, biases, confidences,
                  W1, b1, W2, b2):
    f32 = np.float32
    W1 = np.asarray(W1, dtype=f32)
    # [r, kt*1024 + c] = W1[kt*128 + r, c]
    w1p_h = np.ascontiguousarray(
        W1.reshape(NKT, 128, C).transpose(1, 0, 2).reshape(128, NKT * C))
    combp_h = np.zeros((128, NKT, NST), dtype=f32)
    trigT = np.asarray(triggers, dtype=f32).T.reshape(8, 128, P)  # [kt, r, p]
    ctxT = np.asarray(context, dtype=f32).T.reshape(8, 128, B)    # [kt, r, b]
    for kt in range(8):
        combp_h[:, kt, 100:108] = ctxT[kt]
        combp_h[:, 8 + kt, 0:100] = trigT[kt]
    combp_h = np.ascontiguousarray(combp_h.reshape(128, NKT * NST))
    b1c_h = np.ascontiguousarray(np.asarray(b1, dtype=f32).reshape(8, 128).T)
    w2r_h = np.ascontiguousarray(np.asarray(W2, dtype=f32).reshape(8, 128).T)
    conf_h = np.ascontiguousarray(np.asarray(confidences, dtype=f32).reshape(1, P))
    b2_h = np.ascontiguousarray(np.asarray(b2, dtype=f32).reshape(1, 1))
    attention_scores = np.asarray(attention_scores, dtype=f32)
    biases = np.asarray(biases, dtype=f32)
    in_maps = []
    for r in range(NCORES):
        rows = slice(r * ROWS, (r + 1) * ROWS)
        in_maps.append({
            "bias_s": np.ascontiguousarray(biases[NPE:, rows, :]),
            "biasp_s": np.ascontiguousarray(biases[:NPE, rows, :]),
            "attn_s": np.ascontiguousarray(attention_scores[:, rows, :]),
            "w1p": w1p_h,
            "combp": combp_h,
            "b1c": b1c_h,
            "w2r": w2r_h,
            "conf": conf_h,
            "b2": b2_h,
        })
    return in_maps


def run(trace=False, **inputs):
    nc = _get_nc()
    in_maps = _prep_in_maps(**inputs)
    res = run_bass_kernel_spmd(nc, in_maps, core_ids=list(range(NCORES)),
                               trace=trace)
    out = np.concatenate([np.asarray(res.results[r]["out_s"])
                          for r in range(NCORES)], axis=1)
    return out.astype(np.float32), res


def kernel(**inputs) -> np.ndarray:
    out, _ = run(trace=False, **inputs)
    return out
